# revision 1
# baseline (speedup 1.0000x reference)
"""Causal self-attention Trainium2 kernel (8 NeuronCores, SPMD).

Problem (hardcoded): B=2, T=2048, C=1024, H=16, D=64.
  qkv = x @ W_attn + b_attn ; causal softmax attention ; y @ W_out + b_out

Sharding: core c handles batch b = c//4 and head group g = c%4 (4 heads,
256 channels). Each core computes its heads' attention output and a
partial out-projection [2048, 1024]; the host sums the 4 partials per
batch and adds b_out.

Operands are bf16 (HW-measured ~10% faster per matmul than f32r since
bf16 enables fast-weight-load, and it halves DMA traffic); accumulation
stays fp32 in PSUM. Partial outputs are stored bf16 and upcast on host
(rel err ~4e-3 vs the 2e-2 gate).
Layouts are chosen so no on-device transposes are needed:
  - x arrives transposed ([C, T]) from the host.
  - Q^T/K^T are produced directly in [D, T] (heads paired on 128
    partitions) by using W as the stationary operand.
  - scores are computed transposed (S^T[t, q]) so the softmax sum comes
    free from an appended ones-column on V ([V|1] trick), and exp(S^T)
    blocks feed att@V as the moving operand directly.
  - att@V produces O^T[d, q]; normalization multiplies by the
    broadcast reciprocal of the rowsum row (K=1 matmul broadcast).
"""

import sys

if "/opt/trn_rl_repo" not in sys.path:
    sys.path.insert(0, "/opt/trn_rl_repo")

import numpy as np
import ml_dtypes

BF = ml_dtypes.bfloat16

import concourse.bass as bass
import concourse.mybir as mybir
import concourse.tile as tile
from concourse import bacc, bass_utils

F32 = mybir.dt.float32
F32R = mybir.dt.float32r
BF16 = mybir.dt.bfloat16
MULT = mybir.AluOpType.mult
EXP = mybir.ActivationFunctionType.Exp

B, T, C = 2, 2048, 1024
H, D = 16, 64
HPC = 4          # heads per core
GC = HPC * D     # channels per core's head group (256)
NT = T // 128    # 16 t-tiles
NK = C // 128    # 8 contraction tiles
QCH = 512        # q-chunk width
SCALE = float(1.0 / np.sqrt(D))

_CACHE = {}


def _build(iters=1, phases=3, ablate=None):
    nc = bacc.Bacc("TRN2", target_bir_lowering=False, debug=False,
                   enable_asserts=False, num_devices=8)
    xt_d = nc.dram_tensor("xt", [C, T], BF16, kind="ExternalInput").ap()
    wq_d = nc.dram_tensor("wq", [C, GC], BF16, kind="ExternalInput").ap()
    wk_d = nc.dram_tensor("wk", [C, GC], BF16, kind="ExternalInput").ap()
    wv_d = nc.dram_tensor("wv", [C, GC], BF16, kind="ExternalInput").ap()
    bqk_d = nc.dram_tensor("bqk", [128, 4], F32, kind="ExternalInput").ap()
    bv_d = nc.dram_tensor("bv", [128, GC], F32, kind="ExternalInput").ap()
    wo_d = nc.dram_tensor("wo", [GC, C], BF16, kind="ExternalInput").ap()
    mask_d = nc.dram_tensor("mask", [128, 128], BF16, kind="ExternalInput").ap()
    ones1_d = nc.dram_tensor("ones1", [1, D], F32, kind="ExternalInput").ap()
    onesv_d = nc.dram_tensor("onesv", [128, NT, HPC, 1], BF16, kind="ExternalInput").ap()
    y_d = nc.dram_tensor("y", [T, C], BF16, kind="ExternalOutput").ap()

    import contextlib

    with tile.TileContext(nc) as tc, nc.allow_low_precision(reason="f32r is 32-bit"):
        loop_ctx = tc.For_i(0, iters, 1) if iters > 1 else contextlib.nullcontext()
        with loop_ctx, tc.tile_pool(name="persist", bufs=1) as sb:
            xt = sb.tile([128, NK, T], BF16)
            wq = sb.tile([128, NK, GC], BF16)
            wk = sb.tile([128, NK, GC], BF16)
            wv = sb.tile([128, NK, GC], BF16)
            bqk = sb.tile([128, 4], F32)
            bv = sb.tile([128, GC], F32)
            wo = sb.tile([128, GC // 128, C], BF16)
            mask = sb.tile([128, 128], BF16)
            ones1 = sb.tile([1, D], F32R)
            qt = [sb.tile([128, T], BF16, name=f"qt{i}") for i in range(2)]
            kt = [sb.tile([128, T], BF16, name=f"kt{i}") for i in range(2)]
            vs = sb.tile([128, NT, HPC, D + 1], BF16)
            ot = [sb.tile([128, T], BF16, name=f"ot{i}") for i in range(2)]

            nc.sync.dma_start(out=wq, in_=wq_d.rearrange("(k p) d -> p k d", p=128))
            nc.sync.dma_start(out=wk, in_=wk_d.rearrange("(k p) d -> p k d", p=128))
            xt_src = xt_d.rearrange("(k p) t -> p k t", p=128)
            for k in range(NK):
                nc.sync.dma_start(out=xt[:, k, :], in_=xt_src[:, k, :])
            nc.sync.dma_start(out=wv, in_=wv_d.rearrange("(k p) d -> p k d", p=128))
            nc.sync.dma_start(out=bqk, in_=bqk_d)
            nc.sync.dma_start(out=bv, in_=bv_d)
            nc.sync.dma_start(out=wo, in_=wo_d.rearrange("(k p) e -> p k e", p=128))
            nc.sync.dma_start(out=mask, in_=mask_d)
            nc.sync.dma_start(out=ones1, in_=ones1_d.bitcast(F32R))
            nc.sync.dma_start(out=vs[:, :, :, D:D + 1], in_=onesv_d)

            # ---- Phase 1 + 2: QKV projections and attention, overlapped ----
            def proj_qkt_chunk(ps1, half, n):
                for w_sb, b_col, dst in ((wq, half, qt[half]), (wk, 2 + half, kt[half])):
                    acc = ps1.tile([128, QCH], F32, tag="acc")
                    for k in range(NK):
                        nc.tensor.matmul(
                            acc,
                            lhsT=w_sb[:, k, 128 * half:128 * (half + 1)],
                            rhs=xt[:, k, QCH * n:QCH * (n + 1)],
                            start=(k == 0), stop=(k == NK - 1))
                    nc.vector.tensor_scalar_add(
                        out=dst[:, QCH * n:QCH * (n + 1)], in0=acc,
                        scalar1=bqk[:, b_col:b_col + 1])

            def proj_qt_kt(ps1, half):
                for n in range(T // QCH):
                    proj_qkt_chunk(ps1, half, n)

            # Attention chunk, processed jointly for a HEAD PAIR: both heads'
            # ST matmuls for strip j go back-to-back (they sit on disjoint
            # 64-row PE row-groups, so they run concurrently, and they write
            # different PSUM banks of one shared two-bank tile). One ACT exp
            # covers both heads' strips, amortizing ACT's 352-cycle per-op
            # overhead. The att@V matmuls lag the ST/exp stream by one strip
            # so the in-order PE stream doesn't stall on exp.
            def attn_strips_pair(pools, hp, m0, pending):
                # `pending` carries un-emitted att@V work across chunk
                # boundaries so the PE stream never drains at a chunk end.
                pt_pool, nrm_pool, ps_st, ps_ot, ps_bc = pools
                half = hp
                heads = (2 * hp, 2 * hp + 1)
                q0 = 128 * m0
                ots = [ps_ot.tile([D + 1, QCH], F32, tag="ot", name=f"psum_ot{i}")
                       for i in range(2)]

                def strip_w(j):
                    return QCH - ((j - m0) * 128 if j > m0 else 0)

                for j in range(m0 + QCH // 128):
                    w = strip_w(j)
                    psum_st = ps_st.tile([128, 2 * QCH], F32, tag="st",
                                         name="psum_st")
                    for idx, h in enumerate(heads):
                        poff = 64 * (h % 2)
                        nc.tensor.matmul(
                            psum_st[:, QCH * idx:QCH * idx + w],
                            lhsT=kt[half][poff:poff + D, 128 * j:128 * (j + 1)],
                            rhs=qt[half][poff:poff + D, q0 + QCH - w:q0 + QCH],
                            start=True, stop=True)
                    # one exp over both heads' strips (covers any dead gap
                    # between them; those columns are never read downstream)
                    span = QCH + w
                    pt = pt_pool.tile([128, 2 * QCH], BF16, tag="pt", name="pt")
                    if ablate == "noexp":
                        nc.vector.tensor_copy(out=pt[:, 0:span], in_=psum_st[:, 0:span])
                    else:
                        nc.scalar.activation(out=pt[:, 0:span], in_=psum_st[:, 0:span],
                                             func=EXP, scale=SCALE)
                    if j >= m0:
                        for idx in range(2):
                            nc.vector.tensor_tensor(
                                out=pt[:, QCH * idx:QCH * idx + 128],
                                in0=pt[:, QCH * idx:QCH * idx + 128],
                                in1=mask, op=MULT)
                    pending.append((hp, j, pt, ots, m0))
                    if len(pending) > 2:
                        emit_attv(*pending.pop(0))
                return [(ots[0], half, 0, q0), (ots[1], half, 64, q0)]

            def emit_attv(hp, j, pt, ots_, m0_):
                # One full-width matmul per head: `stop` is sim-only, so no
                # need to split out the diagonal region (an N=128 matmul
                # would run at 4x cost under f32r).
                sb_off = (j - m0_) * 128 if j > m0_ else 0
                w = QCH - sb_off
                last = (j == m0_ + QCH // 128 - 1)
                for idx, h in enumerate((2 * hp, 2 * hp + 1)):
                    off = QCH * idx
                    nc.tensor.matmul(
                        ots_[idx][:, sb_off:QCH],
                        lhsT=vs[:, j, h, :], rhs=pt[:, off:off + w],
                        start=(j == 0), stop=last, skip_group_check=True)

            def attn_flush(pending):
                for args in pending:
                    emit_attv(*args)
                pending.clear()

            def attn_norm(pools, state):
                pt_pool, nrm_pool, ps_st, ps_ot, ps_bc = pools
                psum_ot, half, poff, q0 = state
                if ablate == "nonorm":
                    nc.vector.tensor_copy(out=ot[half][poff:poff + D, q0:q0 + QCH],
                                          in_=psum_ot[0:D, :])
                    return
                rs_recip = nrm_pool.tile([1, QCH], F32R, tag="rs", name="rs_recip")
                nc.vector.reciprocal(out=rs_recip, in_=psum_ot[D:D + 1, :])
                psum_bc = ps_bc.tile([D, QCH], F32, tag="bc", name="psum_bc")
                nc.tensor.matmul(psum_bc, lhsT=ones1, rhs=rs_recip,
                                 start=True, stop=True)
                bc_sb = nrm_pool.tile([D, QCH], F32, tag="bcs", name="bc_sb")
                nc.vector.tensor_copy(out=bc_sb, in_=psum_bc)
                nc.vector.tensor_tensor(
                    out=ot[half][poff:poff + D, q0:q0 + QCH],
                    in0=psum_ot[0:D, :], in1=bc_sb, op=MULT)

            def outproj_block(ps_mm, ystage, m0):
                for i in range(m0, m0 + QCH // 128):
                    for n in range(C // QCH):
                        acc = ps_mm.tile([128, QCH], F32, tag="acc", name="acc")
                        for half in range(2):
                            nc.tensor.matmul(
                                acc,
                                lhsT=ot[half][:, 128 * i:128 * (i + 1)],
                                rhs=wo[:, half, QCH * n:QCH * (n + 1)],
                                start=(half == 0), stop=(half == 1))
                        yt = ystage.tile([128, QCH], BF16, tag="yt", name="yt")
                        nc.vector.tensor_copy(out=yt, in_=acc)
                        nc.sync.dma_start(
                            out=y_d[128 * i:128 * (i + 1), QCH * n:QCH * (n + 1)],
                            in_=yt)

            with tc.tile_pool(name="ps_mm", bufs=1, space="PSUM") as ps_mm, \
                 tc.tile_pool(name="ystage", bufs=2) as ystage:
                if phases < 2:
                    proj_qt_kt(ps_mm, 0)

                def vproj(j):
                    accv = ps_mm.tile([128, GC], F32, tag="acc", name="accv")
                    for k in range(NK):
                        nc.tensor.matmul(
                            accv,
                            lhsT=xt[:, k, 128 * j:128 * (j + 1)],
                            rhs=wv[:, k, :],
                            start=(k == 0), stop=(k == NK - 1))
                    nc.vector.tensor_tensor(
                        out=vs[:, j, :, 0:D],
                        in0=accv.rearrange("p (h d) -> p h d", h=HPC),
                        in1=bv.rearrange("p (h d) -> p h d", h=HPC),
                        op=mybir.AluOpType.add)

                if phases < 2:
                    for j in range(NT):
                        vproj(j)
                with tc.tile_pool(name="pt_pool", bufs=4) as pt_pool, \
                     tc.tile_pool(name="nrm_pool", bufs=2) as nrm_pool, \
                     tc.tile_pool(name="ps_st", bufs=2, space="PSUM") as ps_st, \
                     tc.tile_pool(name="ps_ot", bufs=2, space="PSUM") as ps_ot, \
                     tc.tile_pool(name="ps_bc", bufs=1, space="PSUM") as ps_bc:
                    pools = (pt_pool, nrm_pool, ps_st, ps_ot, ps_bc)
                    # pair 0 attention (emitted before half-1 proj so it
                    # takes PE priority as soon as deps are ready; half-1
                    # proj fills PE gaps while ACT/DVE work on pair 0).
                    # Each task's normalizes are deferred past the next
                    # task's strips to keep the PE stream stall-free.
                    if phases >= 2:
                        prev = None
                        pending = []
                        for m0 in range(0, NT, QCH // 128):
                            # Chunk m0 needs exactly qt/kt column-chunk m0/4
                            # and V tiles m0..m0+3; emitting them here keeps
                            # PE dense while letting ACT start exp almost
                            # immediately instead of idling through the
                            # whole projection.
                            proj_qkt_chunk(ps_mm, 0, m0 // (QCH // 128))
                            for j in range(m0, m0 + QCH // 128):
                                vproj(j)
                            states = attn_strips_pair(pools, 0, m0, pending)
                            # half-1 projection chunks ride along as PE
                            # filler while ACT chews on pair-0 exp work
                            proj_qkt_chunk(ps_mm, 1, m0 // (QCH // 128))
                            if prev is not None:
                                for st_ in prev:
                                    attn_norm(pools, st_)
                            prev = states
                        attn_flush(pending)
                        for st_ in prev:
                            attn_norm(pools, st_)
                    if phases < 2:
                        proj_qt_kt(ps_mm, 1)
                    if phases >= 2:
                        prev = None
                        prev_m0 = None
                        pending = []
                        for m0 in range(0, NT, QCH // 128):
                            states = attn_strips_pair(pools, 1, m0, pending)
                            if prev is not None:
                                for st_ in prev:
                                    attn_norm(pools, st_)
                            if phases >= 3 and prev_m0 is not None:
                                outproj_block(ps_mm, ystage, prev_m0)
                            prev = states
                            prev_m0 = m0
                        attn_flush(pending)
                        for st_ in prev:
                            attn_norm(pools, st_)
                        if phases >= 3:
                            outproj_block(ps_mm, ystage, prev_m0)
    nc.compile()
    return nc


def _get_nc():
    if "nc" not in _CACHE:
        _CACHE["nc"] = _build()
    return _CACHE["nc"]


def make_in_maps(x, W_attn, b_attn, W_out):
    """Per-core input dicts for the SPMD kernel."""
    x = np.asarray(x, dtype=np.float32)
    W_attn = np.asarray(W_attn, dtype=np.float32)
    b_attn = np.asarray(b_attn, dtype=np.float32)
    W_out = np.asarray(W_out, dtype=np.float32)
    mask = np.triu(np.ones((128, 128), np.float32))
    ones1 = np.ones((1, D), np.float32)
    onesv = np.ones((128, NT, HPC, 1), np.float32)
    in_maps = []
    for c in range(8):
        b, g = divmod(c, 4)
        sl = slice(g * GC, (g + 1) * GC)
        bq = b_attn[0 * C:][sl].reshape(2, 128).T          # [128, 2] halves
        bk = b_attn[1 * C:][sl].reshape(2, 128).T
        bqk = np.ascontiguousarray(
            np.stack([bq[:, 0], bq[:, 1], bk[:, 0], bk[:, 1]], axis=1))
        bv = np.tile(b_attn[2 * C:][sl][None, :], (128, 1))
        in_maps.append({
            "xt": np.ascontiguousarray(x[b].T).astype(BF),
            "wq": np.ascontiguousarray(W_attn[:, 0 * C:][:, sl]).astype(BF),
            "wk": np.ascontiguousarray(W_attn[:, 1 * C:][:, sl]).astype(BF),
            "wv": np.ascontiguousarray(W_attn[:, 2 * C:][:, sl]).astype(BF),
            "bqk": bqk,
            "bv": np.ascontiguousarray(bv),
            "wo": np.ascontiguousarray(W_out[sl, :]).astype(BF),
            "mask": mask.astype(BF),
            "ones1": ones1,
            "onesv": onesv.astype(BF),
        })
    return in_maps


def assemble(results, b_out):
    """Sum per-core partials into the full [B, T, C] output."""
    y = np.zeros((B, T, C), np.float32)
    for c in range(8):
        y[c // 4] += results[c]["y"].astype(np.float32)
    y += np.asarray(b_out, dtype=np.float32)[None, None, :]
    return y


def kernel(x, W_attn, b_attn, W_out, b_out):
    nc = _get_nc()
    in_maps = make_in_maps(x, W_attn, b_attn, W_out)
    res = bass_utils.run_bass_kernel_spmd(nc, in_maps, core_ids=list(range(8)))
    return assemble(res.results, b_out)



# revision 68
# speedup vs baseline: 1.0537x; 1.0537x over previous
"""Causal self-attention Trainium2 kernel (8 NeuronCores, SPMD).

Problem (hardcoded): B=2, T=2048, C=1024, H=16, D=64.
  qkv = x @ W_attn + b_attn ; causal softmax attention ; y @ W_out + b_out

Sharding: core c handles batch b = c//4 and head group g = c%4 (4 heads,
256 channels). Each core computes its heads' attention output and a
partial out-projection [2048, 1024]; the host sums the 4 partials per
batch and adds b_out.

Operands are bf16 (fast weight load + half the DMA traffic); accumulation
stays fp32 in PSUM. Partial outputs are stored bf16 and upcast on host
(rel err ~4e-3 vs the 2e-2 gate).
Layouts are chosen so no on-device transposes are needed:
  - x arrives transposed ([C, T]) from the host.
  - Q^T/K^T are produced directly in [D, T] (heads paired on 128
    partitions) by using W as the stationary operand.
  - scores are computed transposed (S^T[t, q]) so the softmax sum comes
    free from an appended ones-column on V ([V|1] trick), and exp(S^T)
    blocks feed att@V as the moving operand directly.
  - att@V produces O^T[d, q]; normalization multiplies by the
    reciprocal of the rowsum row, broadcast across partitions by an
    SBUF->SBUF DMA (keeps the broadcast off the PE).

Schedule notes (PE is the bottleneck engine at ~113us of matmul):
  - input DMAs are issued per-k-tile, interleaved (wq_k, wk_k, xt_k,
    wv_k) so the first projection matmul starts after ~1.1MB instead of
    after the whole 6MB preload.
  - head-pair 1 processes its q-chunks in reverse (12, 8, 4, 0) so the
    kernel tail is the 4-strip chunk, not the 16-strip one (the last
    chunk's exp chain is the serial tail).
  - within a strip, head 1's scores land at column offset w (not QCH),
    so the exp span is exactly 2w with no dead gap.
"""

import sys

if "/opt/trn_rl_repo" not in sys.path:
    sys.path.insert(0, "/opt/trn_rl_repo")

import numpy as np
import ml_dtypes

BF = ml_dtypes.bfloat16

import concourse.bass as bass
import concourse.mybir as mybir
import concourse.tile as tile
from concourse import bacc, bass_utils

F32 = mybir.dt.float32
F32R = mybir.dt.float32r
BF16 = mybir.dt.bfloat16
MULT = mybir.AluOpType.mult
EXP = mybir.ActivationFunctionType.Exp

B, T, C = 2, 2048, 1024
H, D = 16, 64
HPC = 4          # heads per core
GC = HPC * D     # channels per core's head group (256)
NT = T // 128    # 16 t-tiles
NK = C // 128    # 8 contraction tiles
QCH = 512        # q-chunk width
SCALE = float(1.0 / np.sqrt(D))

_CACHE = {}

# HW-bisect flags
MASK_POOL = True    # mask multiply on the Pool/GPSIMD engine (else DVE)
EVAC_ACT = True     # tail outproj evacuation alternates onto ACT


def _build(iters=1, phases=3, ablate=None, sched_n=8):
    nc = bacc.Bacc("TRN2", target_bir_lowering=False, debug=False,
                   enable_asserts=False, num_devices=8)
    xt_d = nc.dram_tensor("xt", [C, T], BF16, kind="ExternalInput").ap()
    wq_d = nc.dram_tensor("wq", [C, GC], BF16, kind="ExternalInput").ap()
    wk_d = nc.dram_tensor("wk", [C, GC], BF16, kind="ExternalInput").ap()
    wv_d = nc.dram_tensor("wv", [C, GC], BF16, kind="ExternalInput").ap()
    bqk_d = nc.dram_tensor("bqk", [128, 4], F32, kind="ExternalInput").ap()
    bv_d = nc.dram_tensor("bv", [128, GC], F32, kind="ExternalInput").ap()
    wo_d = nc.dram_tensor("wo", [GC, C], BF16, kind="ExternalInput").ap()
    mask_d = nc.dram_tensor("mask", [128, 128], BF16, kind="ExternalInput").ap()
    ones1_d = nc.dram_tensor("ones1", [1, D], F32, kind="ExternalInput").ap()
    onesv_d = nc.dram_tensor("onesv", [128, NT, HPC, 1], BF16, kind="ExternalInput").ap()
    y_d = nc.dram_tensor("y", [T, C], BF16, kind="ExternalOutput").ap()

    import contextlib

    with tile.TileContext(nc) as tc, nc.allow_low_precision(reason="f32r is 32-bit"):
        loop_ctx = tc.For_i(0, iters, 1) if iters > 1 else contextlib.nullcontext()
        with loop_ctx, tc.tile_pool(name="persist", bufs=1) as sb:
            xt = sb.tile([128, NK, T], BF16)
            wq = sb.tile([128, NK, GC], BF16)
            wk = sb.tile([128, NK, GC], BF16)
            wv = sb.tile([128, NK, GC], BF16)
            bqk = sb.tile([128, 4], F32)
            bv = sb.tile([128, GC], F32)
            wo = sb.tile([128, GC // 128, C], BF16)
            mask = sb.tile([128, 128], BF16)
            ones1 = sb.tile([1, D], F32R)
            qt = [sb.tile([128, T], BF16, name=f"qt{i}") for i in range(2)]
            kt = [sb.tile([128, T], BF16, name=f"kt{i}") for i in range(2)]
            vs = sb.tile([128, NT, HPC, D + 1], BF16)
            ot = [sb.tile([128, T], BF16, name=f"ot{i}") for i in range(2)]

            # HWDGE descriptor-gen and the HBM transfers are effectively
            # serialized across all queues, so queue-spreading doesn't help;
            # what matters is PRIORITY ORDER (first-needed first). All inputs
            # go on the SP queue, ordered by first use; the first column
            # block of x is split in two so the k0..k3 projection matmuls
            # can start after ~0.6MB instead of ~1MB.
            xt_src = xt_d.rearrange("(k p) t -> p k t", p=128)
            wk_src = wk_d.rearrange("(k p) d -> p k d", p=128)
            nc.sync.dma_start(out=wk[:, 0:4, :], in_=wk_src[:, 0:4, :])
            nc.sync.dma_start(out=xt[:, :, 0:QCH], in_=xt_src[:, :, 0:QCH])
            nc.sync.dma_start(out=wk[:, 4:8, :], in_=wk_src[:, 4:8, :])
            nc.sync.dma_start(out=wv, in_=wv_d.rearrange("(k p) d -> p k d", p=128))
            nc.sync.dma_start(out=bqk, in_=bqk_d)
            nc.sync.dma_start(out=bv, in_=bv_d)
            nc.sync.dma_start(out=wq, in_=wq_d.rearrange("(k p) d -> p k d", p=128))
            nc.sync.dma_start(out=xt[:, :, QCH:2 * QCH],
                              in_=xt_src[:, :, QCH:2 * QCH])
            nc.sync.dma_start(out=mask, in_=mask_d)
            nc.sync.dma_start(out=ones1, in_=ones1_d.bitcast(F32R))
            nc.sync.dma_start(out=vs[:, :, :, D:D + 1], in_=onesv_d)
            for n in range(2, T // QCH):
                nc.sync.dma_start(out=xt[:, :, QCH * n:QCH * (n + 1)],
                                  in_=xt_src[:, :, QCH * n:QCH * (n + 1)])
            nc.sync.dma_start(out=wo, in_=wo_d.rearrange("(k p) e -> p k e", p=128))

            # ---- Phase 1 + 2: QKV projections and attention, overlapped ----
            # Work outside the attention strips is packaged into "units"
            # (one PSUM accumulation group each, ~0.9-1.7us of PE) that the
            # strip loop interleaves into the PE stream as filler, so the
            # in-order PE queue always has runnable matmuls while ACT/DVE
            # chains (exp, norm) resolve.
            def proj_unit(ps1, half, n, which):
                w_sb, b_col, dst = ((wq, half, qt[half]) if which == 0
                                    else (wk, 2 + half, kt[half]))

                def run():
                    acc = ps1.tile([128, QCH], F32, tag="acc")
                    for k in range(NK):
                        nc.tensor.matmul(
                            acc,
                            lhsT=w_sb[:, k, 128 * half:128 * (half + 1)],
                            rhs=xt[:, k, QCH * n:QCH * (n + 1)],
                            start=(k == 0), stop=(k == NK - 1))
                    nc.vector.tensor_scalar_add(
                        out=dst[:, QCH * n:QCH * (n + 1)], in0=acc,
                        scalar1=bqk[:, b_col:b_col + 1])
                return run

            def proj_qkt_chunk(ps1, half, n):
                proj_unit(ps1, half, n, 0)()
                proj_unit(ps1, half, n, 1)()

            def proj_qt_kt(ps1, half):
                for n in range(T // QCH):
                    proj_qkt_chunk(ps1, half, n)

            # Attention chunk, processed jointly for a HEAD PAIR: both heads'
            # ST matmuls for strip j go back-to-back into one shared PSUM
            # tile (head 1 at column offset w so the exp span is 2w, no dead
            # gap). One ACT exp covers both heads' strips. The att@V matmuls
            # lag the ST/exp stream so the in-order PE stream doesn't stall
            # on exp.
            def attn_strips_pair(pools, hp, m0, pending, filler=None,
                                 norms=None):
                # `pending` carries un-emitted att@V work across chunk
                # boundaries so the PE stream never drains at a chunk end.
                # `filler` units are spread evenly across the strips.
                # `norms` is the (to_recip, to_mult) pipeline: the previous
                # chunk's reciprocals are emitted at strip 3 (after its last
                # pending att@V drained) and its multiplies at strip 5, so
                # the broadcast DMA latency sits under ~2 strips of PE work
                # and the waiting ops never head-of-line-block DVE.
                pt_pool, nrm_pool, ps_st, ps_ot, ps_bc = pools
                half = hp
                heads = (2 * hp, 2 * hp + 1)
                q0 = 128 * m0
                ots = [ps_ot.tile([D + 1, QCH], F32, tag="ot", name=f"psum_ot{i}")
                       for i in range(2)]

                def strip_w(j):
                    return QCH - ((j - m0) * 128 if j > m0 else 0)

                n_strips = m0 + QCH // 128
                for j in range(n_strips):
                    w = strip_w(j)
                    # head 1's scores always land in the second PSUM bank:
                    # two matmul accumulation groups must not share a bank,
                    # and crossing the 512-f32 bank boundary is also invalid.
                    off1 = QCH
                    psum_st = ps_st.tile([128, 2 * QCH], F32, tag="st",
                                         name="psum_st")
                    for idx, h in enumerate(heads):
                        poff = 64 * (h % 2)
                        nc.tensor.matmul(
                            psum_st[:, off1 * idx:off1 * idx + w],
                            lhsT=kt[half][poff:poff + D, 128 * j:128 * (j + 1)],
                            rhs=qt[half][poff:poff + D, q0 + QCH - w:q0 + QCH],
                            start=True, stop=True)
                    pt = pt_pool.tile([128, 2 * QCH], BF16, tag="pt", name="pt")
                    # one exp covers both heads when packed back-to-back;
                    # the bank-aligned (w=384) case gets one op per head so
                    # nothing reads the dead gap
                    spans = ([(0, 2 * w)] if off1 == w
                             else [(0, w), (off1, off1 + w)])
                    for lo, hi in spans:
                        if ablate == "noexp":
                            nc.vector.tensor_copy(out=pt[:, lo:hi],
                                                  in_=psum_st[:, lo:hi])
                        else:
                            nc.scalar.activation(out=pt[:, lo:hi],
                                                 in_=psum_st[:, lo:hi],
                                                 func=EXP, scale=SCALE)
                    if j >= m0:
                        eng = nc.gpsimd if MASK_POOL else nc.vector
                        for idx in range(2):
                            eng.tensor_tensor(
                                out=pt[:, off1 * idx:off1 * idx + 128],
                                in0=pt[:, off1 * idx:off1 * idx + 128],
                                in1=mask, op=MULT)
                    pending.append((hp, j, w, off1, pt, ots, m0))
                    if len(pending) > 3:
                        emit_attv(*pending.pop(0))
                    if norms is not None:
                        to_recip, to_mult = norms
                        if j == 3 and to_recip:
                            st = to_recip.popleft()
                            # at a 4-strip loop this is the last strip and
                            # the mult drains immediately below: use the PE
                            # broadcast (no DMA latency in the chain)
                            attn_norm_recip(pools, st,
                                            use_pe=(j == n_strips - 1))
                            to_mult.append(st)
                        if j == 5 or j == n_strips - 1:
                            while to_mult:
                                attn_norm_mult(pools, to_mult.popleft())
                    if filler and j > 0:
                        for _ in range(-(-len(filler) // (n_strips - j))):
                            filler.popleft()()
                return [ots, half, q0]

            def emit_attv(hp, j, w, off1, pt, ots_, m0_):
                last = (j == m0_ + QCH // 128 - 1)
                sb_off = QCH - w
                for idx, h in enumerate((2 * hp, 2 * hp + 1)):
                    nc.tensor.matmul(
                        ots_[idx][:, sb_off:QCH],
                        lhsT=vs[:, j, h, :], rhs=pt[:, off1 * idx:off1 * idx + w],
                        start=(j == 0), stop=last, skip_group_check=True)

            def attn_flush(pending):
                for args in pending:
                    emit_attv(*args)
                pending.clear()

            def attn_norm_recip(pools, state, use_pe=True):
                # Reciprocal rows are broadcast across partitions by two K=1
                # f32r matmuls (512 cycles each - the only PE cost of the
                # whole normalization) and evacuated by DVE copies.
                pt_pool, nrm_pool, ps_st, ps_ot, ps_bc = pools
                ots, half, q0 = state[:3]
                if ablate == "nonorm":
                    state.append(None)
                    return
                bc = nrm_pool.tile([128, QCH], F32, tag="bc", name="bc_sb")
                for idx in range(2):
                    rs1 = nrm_pool.tile([1, QCH], F32R, tag="rs",
                                        name="rs1")
                    nc.vector.reciprocal(out=rs1, in_=ots[idx][D:D + 1, :])
                    pb = ps_bc.tile([128, QCH], F32, tag="acc",
                                    name="ps_bc")
                    nc.tensor.matmul(
                        pb[0:64, :], lhsT=ones1, rhs=rs1,
                        start=True, stop=True)
                    nc.vector.tensor_copy(out=bc[64 * idx:64 * (idx + 1), :],
                                          in_=pb[0:64, :])
                state.append(bc)

            def attn_norm_mult(pools, state):
                ots, half, q0, bc = state
                for idx in range(2):
                    poff = 64 * idx
                    if ablate == "nonorm":
                        nc.vector.tensor_copy(
                            out=ot[half][poff:poff + D, q0:q0 + QCH],
                            in_=ots[idx][0:D, :])
                    else:
                        nc.vector.tensor_tensor(
                            out=ot[half][poff:poff + D, q0:q0 + QCH],
                            in0=ots[idx][0:D, :], in1=bc[poff:poff + 64, :],
                            op=MULT)

            def attn_norm(pools, state):
                attn_norm_recip(pools, state)
                attn_norm_mult(pools, state)

            def outproj_unit(ps_mm, ystage, i, n, evac="dve"):
                def run():
                    acc = ps_mm.tile([128, QCH], F32, tag="acc", name="acc")
                    for half in range(2):
                        nc.tensor.matmul(
                            acc,
                            lhsT=ot[half][:, 128 * i:128 * (i + 1)],
                            rhs=wo[:, half, QCH * n:QCH * (n + 1)],
                            start=(half == 0), stop=(half == 1))
                    yt = ystage.tile([128, QCH], BF16, tag="yt", name="yt")
                    if evac == "act" and EVAC_ACT:
                        # tail blocks: ACT is idle once the last exp is done,
                        # and this keeps the norm chain's DVE ops unblocked.
                        nc.scalar.copy(out=yt, in_=acc)
                    else:
                        nc.vector.tensor_copy(out=yt, in_=acc)
                    nc.sync.dma_start(
                        out=y_d[128 * i:128 * (i + 1), QCH * n:QCH * (n + 1)],
                        in_=yt)
                return run

            def outproj_units(ps_mm, ystage, m0, evac="dve"):
                units = []
                for idx, (i, n) in enumerate(
                        (i, n) for i in range(m0, m0 + QCH // 128)
                        for n in range(C // QCH)):
                    e = ("act" if idx % 2 else "dve") if evac == "alt" else evac
                    units.append(outproj_unit(ps_mm, ystage, i, n, e))
                return units

            def outproj_block(ps_mm, ystage, m0, evac="dve"):
                for u in outproj_units(ps_mm, ystage, m0, evac):
                    u()

            with tc.tile_pool(name="ps_mm", bufs=2, space="PSUM") as ps_mm, \
                 tc.tile_pool(name="ystage", bufs=4) as ystage:
                if phases < 2:
                    proj_qt_kt(ps_mm, 0)

                def vproj_unit(j):
                    def run():
                        accv = ps_mm.tile([128, GC], F32, tag="acc", name="accv")
                        for k in range(NK):
                            nc.tensor.matmul(
                                accv,
                                lhsT=xt[:, k, 128 * j:128 * (j + 1)],
                                rhs=wv[:, k, :],
                                start=(k == 0), stop=(k == NK - 1))
                        nc.vector.tensor_tensor(
                            out=vs[:, j, :, 0:D],
                            in0=accv.rearrange("p (h d) -> p h d", h=HPC),
                            in1=bv.rearrange("p (h d) -> p h d", h=HPC),
                            op=mybir.AluOpType.add)
                    return run

                def vproj(j):
                    vproj_unit(j)()

                if phases < 2:
                    for j in range(NT):
                        vproj(j)
                with tc.tile_pool(name="pt_pool", bufs=6) as pt_pool, \
                     tc.tile_pool(name="nrm_pool", bufs=2) as nrm_pool, \
                     tc.tile_pool(name="ps_st", bufs=2, space="PSUM") as ps_st, \
                     tc.tile_pool(name="ps_ot", bufs=2, space="PSUM") as ps_ot:
                    # the PE-broadcast norm reuses ps_mm's rotating buffers
                    pools = (pt_pool, nrm_pool, ps_st, ps_ot, ps_mm)
                    if phases >= 2:
                        # Unified schedule: the two head-pairs' chunks
                        # interleave, so attention strips (the work ACT/DVE
                        # chains hang off) are spread across the WHOLE
                        # kernel, and every projection / V / out-projection
                        # unit rides as evenly-paced PE filler inside some
                        # strip loop, one loop ahead of its consumer. Chunk
                        # order ends on the 4-strip chunks so the tail exp
                        # chain is short. Norms flow through the
                        # (to_recip, to_mult) pipeline hooks; outproj blocks
                        # join the filler at hand-balanced slots (each is
                        # ready one slot earlier).
                        from collections import deque
                        pending = []
                        filler = deque()
                        norms = (deque(), deque())
                        SCHED = [(0, 4), (1, 4), (0, 8), (1, 8),
                                 (0, 12), (1, 12), (0, 0), (1, 0)]
                        # proj filler additions per schedule slot
                        # (consumed one loop before the consumer's slot)
                        PROJ_ADDS = {
                            0: [(1, 1, 0), (1, 0, 1), (1, 1, 1)],
                            1: [(0, 2, 0), (0, 2, 1)],
                            2: [(1, 2, 0), (1, 2, 1)],
                            3: [(0, 3, 0), (0, 3, 1)],
                            4: [(1, 3, 0), (1, 3, 1)],
                            5: [(0, 0, 0)],
                            6: [(1, 0, 0)],
                        }
                        VPROJ_ADDS = {0: range(8, 10), 1: range(10, 12),
                                      2: range(12, 14), 3: range(14, 16)}
                        OUT_ADDS = {4: (4, "dve"), 5: (8, "dve"),
                                    7: (12, "alt")}
                        # preloop: what (0,4) needs - kt0 chunks 0-1, qt0
                        # chunk 1, V tiles 0-7 - emitted in DMA-arrival order
                        proj_unit(ps_mm, 0, 0, 1)()
                        for j in range(0, 4):
                            vproj(j)
                        proj_unit(ps_mm, 0, 1, 1)()
                        proj_unit(ps_mm, 0, 1, 0)()
                        for j in range(4, 8):
                            vproj(j)
                        for si, (hp, m0) in enumerate(SCHED[:sched_n]):
                            for args in PROJ_ADDS.get(si, []):
                                h, n, which = args
                                filler.append(proj_unit(ps_mm, h, n, which))
                            filler.extend(vproj_unit(j)
                                          for j in VPROJ_ADDS.get(si, []))
                            if phases >= 3 and si in OUT_ADDS:
                                m_out, ev = OUT_ADDS[si]
                                filler.extend(outproj_units(
                                    ps_mm, ystage, m_out, evac=ev))
                            state = attn_strips_pair(pools, hp, m0, pending,
                                                     filler, norms)
                            norms[0].append(state)
                        attn_flush(pending)
                        # Tail: last reciprocal via the PE broadcast (no DMA
                        # in the chain), then the final outproj block.
                        while norms[1]:
                            attn_norm_mult(pools, norms[1].popleft())
                        while norms[0]:
                            st = norms[0].popleft()
                            attn_norm_recip(pools, st, use_pe=True)
                            attn_norm_mult(pools, st)
                        if phases >= 3:
                            outproj_block(ps_mm, ystage, 0, evac="alt")
    nc.compile()
    return nc


def _get_nc():
    if "nc" not in _CACHE:
        _CACHE["nc"] = _build()
    return _CACHE["nc"]


def make_in_maps(x, W_attn, b_attn, W_out):
    """Per-core input dicts for the SPMD kernel."""
    x = np.asarray(x, dtype=np.float32)
    W_attn = np.asarray(W_attn, dtype=np.float32)
    b_attn = np.asarray(b_attn, dtype=np.float32)
    W_out = np.asarray(W_out, dtype=np.float32)
    mask = np.triu(np.ones((128, 128), np.float32))
    ones1 = np.ones((1, D), np.float32)
    onesv = np.ones((128, NT, HPC, 1), np.float32)
    in_maps = []
    for c in range(8):
        b, g = divmod(c, 4)
        sl = slice(g * GC, (g + 1) * GC)
        bq = b_attn[0 * C:][sl].reshape(2, 128).T          # [128, 2] halves
        bk = b_attn[1 * C:][sl].reshape(2, 128).T
        bqk = np.ascontiguousarray(
            np.stack([bq[:, 0], bq[:, 1], bk[:, 0], bk[:, 1]], axis=1))
        bv = np.tile(b_attn[2 * C:][sl][None, :], (128, 1))
        in_maps.append({
            "xt": np.ascontiguousarray(x[b].T).astype(BF),
            "wq": np.ascontiguousarray(W_attn[:, 0 * C:][:, sl]).astype(BF),
            "wk": np.ascontiguousarray(W_attn[:, 1 * C:][:, sl]).astype(BF),
            "wv": np.ascontiguousarray(W_attn[:, 2 * C:][:, sl]).astype(BF),
            "bqk": bqk,
            "bv": np.ascontiguousarray(bv),
            "wo": np.ascontiguousarray(W_out[sl, :]).astype(BF),
            "mask": mask.astype(BF),
            "ones1": ones1,
            "onesv": onesv.astype(BF),
        })
    return in_maps


def assemble(results, b_out):
    """Sum per-core partials into the full [B, T, C] output."""
    y = np.zeros((B, T, C), np.float32)
    for c in range(8):
        y[c // 4] += results[c]["y"].astype(np.float32)
    y += np.asarray(b_out, dtype=np.float32)[None, None, :]
    return y


def kernel(x, W_attn, b_attn, W_out, b_out):
    nc = _get_nc()
    in_maps = make_in_maps(x, W_attn, b_attn, W_out)
    res = bass_utils.run_bass_kernel_spmd(nc, in_maps, core_ids=list(range(8)))
    return assemble(res.results, b_out)
